# revision 29
# baseline (speedup 1.0000x reference)
"""GAT (8-layer, 8-head) Trainium2 Bass kernel, 8-core SPMD — v2.

Empirical cost model of this (axon-tunneled) environment, measured:
per-call wall = dispatch floor (~0.1s) + bytes-through-tunnel (~125MB/s)
+ ~35us per STATIC instruction (per-call program processing); dynamic
(executed) instructions are nearly free, so hardware For_i loops +
minimal I/O dominate everything a classic roofline would suggest.

v2 therefore:
- rolls the per-layer window loop into a hardware For_i (one ~56-instr
  static body instead of 8x20 unrolled copies),
- unrolls the 8 layers (collective_compute inside For_i hangs at runtime),
- phase A emits node-major [h|s] directly (lhsT = x^T chunk stationary),
  with batched PSUM evacuation,
- minimizes shipped bytes: x and out in bf16; only src indices (int16)
  and window-local dst offsets (int8) are shipped, 16-partition-wide,
  replicated on device; dst gather indices and the one-hot dloc table
  are derived on device (each layer copies the own shard to a fixed
  table slab so dst rows are core-independent); weights/biases sharded
  8-ways and reassembled with an AllGather; iota/identity generated on
  device,
- caches the traced jit executable + concatenated inputs across calls and
  generates donated output zeros on device (run_bass_via_pjrt re-traces
  and ships host zeros every call).
Hard limits found empirically: dma_gather >1024 indices per call crashes
(runtime descriptor ring), dma_scatter_add races on duplicate indices,
matmul stationary (ldweights) cannot take register offsets, compute
engines cannot start at partition offsets not multiple of 32.

Math identical to v1: fused [h|s] = x @ [W | W@A2], AllGather full node
table, per dst-window gather rows by src and dst, e=lrelu(s_src+s_dst),
ex=exp(e) (no max subtraction), one-hot scatter via PE matmul
psum[j,72] += S^T [ex*h | ex], out = psum[:,:64]/(psum[:,64:72]+1e-16)+b.
"""

import numpy as np
import ml_dtypes

N_NODES = 20000
N_EDGES = 640000
L, H, C = 8, 8, 8
D = H * C  # 64
NEG_SLOPE = 0.2

NCORES = 8
WIN = 128                 # dst nodes per window
WPC = 20                  # windows per core
NSH = WIN * WPC           # 2560 nodes per shard
NPAD = NCORES * NSH       # 20480
SENT = NPAD               # sentinel node id (table row)
TROWS = NPAD + 1          # table rows (incl. sentinel)

_cache = {}
USE_LOOP = True     # window loop as HW For_i (static-size win)
FAT_GATHER = False  # >1024-idx dma_gather crashes the device; keep chunked
BATCH_PST = True    # batched PSUM evacuation in phase A


# ----------------------------------------------------------------------------
# Host preprocessing
# ----------------------------------------------------------------------------
def _prep_edges(edge_index):
    src = np.asarray(edge_index[0], dtype=np.int64)
    dst = np.asarray(edge_index[1], dtype=np.int64)
    src = np.concatenate([src, np.arange(N_NODES, dtype=np.int64)])
    dst = np.concatenate([dst, np.arange(N_NODES, dtype=np.int64)])
    order = np.argsort(dst, kind="stable")
    src, dst = src[order], dst[order]

    nwin = NCORES * WPC  # 160
    win_of_edge = dst // WIN
    counts = np.bincount(win_of_edge, minlength=nwin)
    tpw = int(np.ceil(counts.max() / 128))
    nsw = tpw * 128                      # slots per window
    nslot = WPC * nsw                    # slots per core

    src_slot = np.full((NCORES, nslot), SENT, dtype=np.int64)
    dst_slot = np.full((NCORES, nslot), SENT, dtype=np.int64)
    dloc_slot = np.full((NCORES, nslot), -1.0, dtype=np.float32)

    wstart = np.zeros(nwin + 1, dtype=np.int64)
    np.cumsum(counts, out=wstart[1:])
    for w in range(nwin):
        c, wl = divmod(w, WPC)
        e0, e1 = wstart[w], wstart[w + 1]
        s0 = wl * nsw
        n = e1 - e0
        src_slot[c, s0:s0 + n] = src[e0:e1]
        dst_slot[c, s0:s0 + n] = dst[e0:e1]
        dloc_slot[c, s0:s0 + n] = (dst[e0:e1] - w * WIN).astype(np.float32)

    def wrap16(a, dt):
        # index i -> [i%16, i//16]; replicated to 128 partitions on device
        return a.reshape(-1, 16).T.astype(dt).copy()   # [16, nslot/16]

    srcidx = np.stack([wrap16(src_slot[c], np.int16) for c in range(NCORES)])
    dl16 = np.stack([wrap16(dloc_slot[c].astype(np.int64), np.int8)
                     for c in range(NCORES)])
    return tpw, srcidx, dl16


# ----------------------------------------------------------------------------
# Bass program
# ----------------------------------------------------------------------------
def _build(tpw):
    import concourse.bass as bass
    import concourse.tile as tile
    import concourse.mybir as mybir
    from concourse import bacc
    from contextlib import ExitStack

    f32 = mybir.dt.float32
    bf16 = mybir.dt.bfloat16
    i16 = mybir.dt.int16
    Alu = mybir.AluOpType
    Act = mybir.ActivationFunctionType

    nsw = tpw * 128
    nslot = WPC * nsw

    nc = bacc.Bacc("TRN2", target_bir_lowering=False, debug=False,
                   num_devices=NCORES)

    # external I/O
    t_xsh = nc.dram_tensor("xsh", [NSH, D], bf16, kind="ExternalInput")
    t_srci = nc.dram_tensor("srcidx", [16, nslot // 16], i16, kind="ExternalInput")
    t_dl16 = nc.dram_tensor("dl16", [16, nslot // 16], mybir.dt.int8,
                            kind="ExternalInput")
    t_wts = nc.dram_tensor("wts", [8, L, 80], f32, kind="ExternalInput")
    t_brep = nc.dram_tensor("brep", [2, L, 64], f32, kind="ExternalInput")
    t_out = nc.dram_tensor("out", [NSH, D], bf16, kind="ExternalOutput")

    with tile.TileContext(nc) as tc, ExitStack() as ctx:
        cpool = ctx.enter_context(tc.tile_pool(name="const", bufs=1))
        gpool = ctx.enter_context(tc.tile_pool(name="gath", bufs=2))
        epool = ctx.enter_context(tc.tile_pool(name="edge", bufs=2))
        dram = ctx.enter_context(tc.tile_pool(name="dram", bufs=1, space="DRAM"))
        psT = ctx.enter_context(tc.tile_pool(name="psT", bufs=2, space="PSUM"))
        psA = ctx.enter_context(tc.tile_pool(name="psA", bufs=2, space="PSUM"))
        psW = ctx.enter_context(tc.tile_pool(name="psW", bufs=2, space="PSUM"))

        # persistent SBUF
        sb_x = cpool.tile([128, WPC, D], f32)          # node-major shard x
        sb_srci = cpool.tile([128, WPC, nsw // 16], i16)
        sb_dl16 = cpool.tile([128, WPC, nsw // 16], mybir.dt.int8)
        sb_dloc = cpool.tile([128, WPC, tpw], bf16)
        sb_iota = cpool.tile([128, 128], bf16)
        sb_ident = cpool.tile([128, 128], f32)
        sb_wts = cpool.tile([64, L, 80], f32)
        sb_brep = cpool.tile([128, L, 64], f32)
        xT = cpool.tile([64, WPC, 128], f32)           # feature-major shard x
        tabsb = cpool.tile([128, WPC, 128], bf16)

        xin = cpool.tile([128, WPC, D], bf16)
        nc.sync.dma_start(xin[:], t_xsh.ap().rearrange("(t p) c -> p t c", p=128))
        nc.vector.tensor_copy(sb_x[:], xin[:])
        for r in range(8):
            nc.sync.dma_start(sb_srci[16 * r:16 * (r + 1), :, :],
                              t_srci.ap().rearrange("p (w s) -> p w s", w=WPC))
            nc.sync.dma_start(sb_dl16[16 * r:16 * (r + 1), :, :],
                              t_dl16.ap().rearrange("p (w s) -> p w s", w=WPC))
        # wrap128 dloc from the wrap16 table:
        # wrap128[16*(q%8)+r, j] = wrap16[r, 8j+(q%8)]  (replicated blocks)
        dl8 = cpool.tile([128, WPC, tpw], mybir.dt.int8)
        dl16v = sb_dl16[:].rearrange("p w (t e) -> p w t e", e=8)
        for pb in range(8):
            nc.sync.dma_start(dl8[16 * pb:16 * (pb + 1), :, :],
                              dl16v[16 * pb:16 * (pb + 1), :, :, pb])
        nc.vector.tensor_copy(sb_dloc[:], dl8[:])
        # iota row (0..127 along free) and identity, generated on device
        irow = cpool.tile([128, 128], i16)
        icol = cpool.tile([128, 128], i16)
        nc.gpsimd.iota(irow[:], pattern=[[1, 128]], base=0, channel_multiplier=0)
        nc.gpsimd.iota(icol[:], pattern=[[0, 128]], base=0, channel_multiplier=1)
        nc.vector.tensor_copy(sb_iota[:], irow[:])
        nc.vector.tensor_tensor(sb_ident[:], irow[:], icol[:], Alu.is_equal)
        # commons are sharded across cores; AllGather reassembles them
        WFULL = dram.tile([64, L, 80], f32)
        BFULL = dram.tile([16, L, 64], f32)
        WSTG = dram.tile([8, L, 80], f32)
        BSTG = dram.tile([2, L, 64], f32)
        nc.sync.dma_start(WSTG[:], t_wts.ap())
        nc.sync.dma_start(BSTG[:], t_brep.ap())
        nc.gpsimd.collective_compute(
            "AllGather", Alu.bypass, replica_groups=[list(range(NCORES))],
            ins=[WSTG[:].opt()], outs=[WFULL[:].opt()])
        nc.gpsimd.collective_compute(
            "AllGather", Alu.bypass, replica_groups=[list(range(NCORES))],
            ins=[BSTG[:].opt()], outs=[BFULL[:].opt()])
        nc.sync.dma_start(sb_wts[:], WFULL[:])
        for r in range(8):
            nc.sync.dma_start(sb_brep[16 * r:16 * (r + 1), :, :], BFULL[:])

        # DRAM: gather table + staging shard. bf16 rows (256B):
        # [0:64] h bf16; f32 view: [32:40] s_src, [40:48] s_dst, [48:64] pad
        TAB = dram.tile([TROWS + NSH, 128], bf16)
        STAGE = dram.tile([NSH, 128], bf16)

        # STAGE bf16 cols 96:128 (f32 48:64) are never produced; zero once
        zjunk = cpool.tile([128, WPC, 32], bf16)
        nc.vector.memset(zjunk[:], 0.0)
        nc.sync.dma_start(
            STAGE[:, 96:128].rearrange("(t p) c -> p t c", p=128), zjunk[:])

        # sentinel row: h=0, s_src=-1e30 (=> ex = 0 for padding), s_dst=0
        sent = cpool.tile([1, 128], bf16)
        nc.vector.memset(sent[:], 0.0)
        nc.vector.memset(sent[:].bitcast(f32)[:, 32:40], -1e30)
        nc.sync.dma_start(TAB[SENT:SENT + 1, :], sent[:])

        # dst rows live in the fixed own-slab at TAB[TROWS + 128*w + dloc]
        iw = cpool.tile([128, WPC], i16)
        nc.gpsimd.iota(iw[:], pattern=[[128, WPC]], base=TROWS,
                       channel_multiplier=0)
        sb_ofsf = cpool.tile([128, WPC, 1], f32)
        nc.vector.tensor_copy(sb_ofsf[:].rearrange("p w e -> p (w e)"), iw[:])

        for l in range(L):
            # ---------------- phase A: per-node prep (own shard) ----------
            # x^T chunks via PE transpose (batched psum evac, 4 per bank)
            if BATCH_PST:
                for t0 in range(0, WPC, 4):
                    pt = psT.tile([64, 4, 128], f32, tag="psT")
                    for j in range(4):
                        nc.tensor.transpose(pt[:, j, :], sb_x[:, t0 + j, :],
                                            sb_ident[:])
                    nc.scalar.copy(xT[:, t0:t0 + 4, :], pt[:])
            else:
                for t in range(WPC):
                    pt = psT.tile([64, 128], f32, tag="psT")
                    nc.tensor.transpose(pt[:], sb_x[:, t, :], sb_ident[:])
                    nc.scalar.copy(xT[:, t, :], pt[:])

            # node-major [h|s] directly: out[128n, 80] = xT_chunk^T @ wts_l
            if BATCH_PST:
                for t0 in range(0, WPC, 5):
                    ph = psA.tile([128, 5, 80], f32, tag="psA")
                    for j in range(5):
                        nc.tensor.matmul(ph[:, j, :], lhsT=xT[:, t0 + j, :],
                                         rhs=sb_wts[:, l, :],
                                         start=True, stop=True)
                    nc.scalar.copy(tabsb[:, t0:t0 + 5, 0:64], ph[:, :, 0:64])
                    nc.vector.tensor_copy(
                        tabsb[:, t0:t0 + 5, :].bitcast(f32)[:, :, 32:48],
                        ph[:, :, 64:80])
            else:
                for t in range(WPC):
                    ph = psA.tile([128, 80], f32, tag="psA")
                    nc.tensor.matmul(ph[:], lhsT=xT[:, t, :],
                                     rhs=sb_wts[:, l, :],
                                     start=True, stop=True)
                    nc.scalar.copy(tabsb[:, t, 0:64], ph[:, 0:64])
                    nc.vector.tensor_copy(
                        tabsb[:, t, :].bitcast(f32)[:, 32:48], ph[:, 64:80])

            nc.sync.dma_start(
                STAGE[:, 0:96].rearrange("(t p) c -> p t c", p=128),
                tabsb[:, :, 0:96])
            nc.sync.dma_start(
                TAB[TROWS:TROWS + NSH, :].rearrange("(t p) c -> p t c", p=128),
                tabsb[:])
            nc.gpsimd.collective_compute(
                "AllGather", Alu.bypass,
                replica_groups=[list(range(NCORES))],
                ins=[STAGE[:].opt()],
                outs=[TAB[0:NPAD, :].opt()],
            )

            # ---------------- phase B: edges, per window (HW loop) --------
            from contextlib import nullcontext

            def _window_body(w):
                vs = gpool.tile([128, tpw, 128], bf16, tag="vsrc")
                vd = gpool.tile([128, tpw, 128], bf16, tag="vdst")
                vdix = epool.tile([128, nsw // 16], i16, tag="vdix")
                nc.vector.tensor_scalar_add(vdix[:], sb_dl16[:, w, :],
                                            sb_ofsf[:, w, :])

                if FAT_GATHER:
                    nc.gpsimd.dma_gather(
                        out_ap=vs[:], in_ap=TAB[:], idxs_ap=sb_srci[:, w, :],
                        num_idxs=nsw, num_idxs_reg=nsw, elem_size=128)
                    nc.gpsimd.dma_gather(
                        out_ap=vd[:], in_ap=TAB[:], idxs_ap=vdix[:],
                        num_idxs=nsw, num_idxs_reg=nsw, elem_size=128)
                else:
                    GCH = 8
                    for u0 in range(0, tpw, GCH):
                        u1 = min(u0 + GCH, tpw)
                        n = (u1 - u0) * 128
                        nc.gpsimd.dma_gather(
                            out_ap=vs[:, u0:u1, :], in_ap=TAB[:],
                            idxs_ap=sb_srci[:, w, u0 * 8:u1 * 8],
                            num_idxs=n, num_idxs_reg=n, elem_size=128)
                        nc.gpsimd.dma_gather(
                            out_ap=vd[:, u0:u1, :], in_ap=TAB[:],
                            idxs_ap=vdix[:, u0 * 8:u1 * 8],
                            num_idxs=n, num_idxs_reg=n, elem_size=128)

                # one-hot S: [128, tpw, 128] bf16
                S = epool.tile([128, tpw, 128], bf16, tag="S")
                nc.vector.tensor_tensor(
                    S[:],
                    sb_dloc[:, w, :].unsqueeze(2).broadcast_to([128, tpw, 128]),
                    sb_iota[:].unsqueeze(1).broadcast_to([128, tpw, 128]),
                    Alu.is_equal)

                # e = lrelu(s_src + s_dst); ex = exp(e)
                e = epool.tile([128, tpw, 8], f32, tag="e")
                nc.vector.tensor_tensor(
                    e[:], vs[:].bitcast(f32)[:, :, 32:40],
                    vd[:].bitcast(f32)[:, :, 40:48], Alu.add)
                nc.vector.scalar_tensor_tensor(e[:], e[:], NEG_SLOPE, e[:],
                                               op0=Alu.mult, op1=Alu.max)
                ex = epool.tile([128, tpw, 8], f32, tag="ex")
                nc.scalar.activation(ex[:], e[:], Act.Exp)
                # R = [V*ex | ex] in bf16
                R = epool.tile([128, tpw, 72], bf16, tag="R")
                nc.vector.tensor_copy(R[:, :, 64:72], ex[:])
                nc.vector.tensor_tensor(
                    R[:, :, 0:64].rearrange("p t (h c) -> p t h c", h=8),
                    vs[:, :, 0:64].rearrange("p t (h c) -> p t h c", h=8),
                    R[:, :, 64:72].unsqueeze(3).broadcast_to(
                        [128, tpw, 8, 8]),
                    Alu.mult)

                pw = psW.tile([128, 72], f32, tag="psW")
                for t in range(tpw):
                    nc.tensor.matmul(pw[:], lhsT=S[:, t, :], rhs=R[:, t, :],
                                     start=(t == 0), stop=(t == tpw - 1))

                # out = psum[:, :64] / (z + 1e-16) + bias
                zi = epool.tile([128, 8], f32, tag="zi")
                nc.vector.tensor_scalar_add(zi[:], pw[:, 64:72], 1e-16)
                rz = epool.tile([128, 8], f32, tag="rz")
                nc.vector.reciprocal(rz[:], zi[:])
                xm = epool.tile([128, 64], f32, tag="xm")
                nc.vector.tensor_tensor(
                    xm[:].rearrange("p (h c) -> p h c", h=8),
                    pw[:, 0:64].rearrange("p (h c) -> p h c", h=8),
                    rz[:].unsqueeze(2).broadcast_to([128, 8, 8]),
                    Alu.mult)
                nc.vector.tensor_tensor(sb_x[:, w, :], xm[:],
                                        sb_brep[:, l, :], Alu.add)

            if USE_LOOP:
                with tc.For_i(0, WPC) as w:
                    _window_body(w)
            else:
                for w in range(WPC):
                    _window_body(w)

        outbf = cpool.tile([128, WPC, D], bf16)
        nc.vector.tensor_copy(outbf[:], sb_x[:])
        nc.sync.dma_start(t_out.ap().rearrange("(t p) c -> p t c", p=128),
                          outbf[:])

    nc.finalize()
    return nc


def _get_program(tpw):
    key = (tpw, USE_LOOP, FAT_GATHER, BATCH_PST)
    if key not in _cache:
        _cache[key] = _build(tpw)
    return _cache[key]


# ----------------------------------------------------------------------------
# Entry point
# ----------------------------------------------------------------------------
def make_program_and_inputs(x, edge_index, Ws, att_src, att_dst, biases):
    x = np.asarray(x, dtype=np.float32)
    Ws = np.asarray(Ws, dtype=np.float32)
    att_src = np.asarray(att_src, dtype=np.float32)
    att_dst = np.asarray(att_dst, dtype=np.float32)
    biases = np.asarray(biases, dtype=np.float32)

    tpw, srcidx, dl16 = _prep_edges(edge_index)
    nc = _get_program(tpw)

    xpad = np.zeros((NPAD, D), np.float32)
    xpad[:N_NODES] = x

    # A2[cout, l, 0:8] = att_src heads, [.., 8:16] = att_dst heads
    a2 = np.zeros((64, L, 16), np.float32)
    for l in range(L):
        for h in range(H):
            a2[h * C:(h + 1) * C, l, h] = att_src[l, h]
            a2[h * C:(h + 1) * C, l, 8 + h] = att_dst[l, h]
    # wts[cin, l, 0:64] = W; [cin, l, 64:80] = W @ A2  (s = x @ (W A2))
    wts = np.zeros((64, L, 80), np.float32)
    for l in range(L):
        wts[:, l, 0:64] = Ws[l]
        wts[:, l, 64:80] = Ws[l] @ a2[:, l, :]
    brep16 = np.broadcast_to(biases[None, :, :], (16, L, 64))
    in_maps = []
    for c in range(NCORES):
        in_maps.append(dict(
            wts=np.ascontiguousarray(wts[8 * c:8 * (c + 1)]),
            brep=np.ascontiguousarray(brep16[2 * c:2 * (c + 1)]),
            xsh=np.ascontiguousarray(
                xpad[c * NSH:(c + 1) * NSH]).astype(ml_dtypes.bfloat16),
            srcidx=srcidx[c], dl16=dl16[c]))
    return nc, in_maps


# ----------------------------------------------------------------------------
# Cached PJRT runner. Mirrors concourse.bass2jax.run_bass_via_pjrt's
# multi-core branch, but (a) caches the traced/jitted executable across
# calls (run_bass_via_pjrt rebuilds closures and re-traces every call) and
# (b) generates the donated zero output buffers ON DEVICE instead of
# shipping host zeros through the axon tunnel each call.
# ----------------------------------------------------------------------------
_runner_cache = {}


def _get_runner(nc, n_cores):
    key = id(nc)
    if key in _runner_cache:
        return _runner_cache[key]
    import jax
    import jax.numpy as jnp
    from jax.experimental.shard_map import shard_map
    from jax.sharding import Mesh, NamedSharding, PartitionSpec
    from concourse import bass2jax as b2j
    import concourse.mybir as mybir

    b2j.install_neuronx_cc_hook()
    partition_name = (nc.partition_id_tensor.name
                      if nc.partition_id_tensor else None)
    in_names, out_names, out_avals = [], [], []
    for alloc in nc.m.functions[0].allocations:
        if not isinstance(alloc, mybir.MemoryLocationSet):
            continue
        name = alloc.memorylocations[0].name
        if alloc.kind == "ExternalInput":
            if name != partition_name:
                in_names.append(name)
        elif alloc.kind == "ExternalOutput":
            shape = tuple(alloc.tensor_shape)
            dtype = mybir.dt.np(alloc.dtype)
            out_names.append(name)
            out_avals.append(jax.core.ShapedArray(shape, dtype))
    n_params = len(in_names)
    n_outs = len(out_names)
    all_in = in_names + out_names + ([partition_name] if partition_name else [])

    def _body(*args):
        operands = list(args)
        if partition_name is not None:
            operands.append(b2j.partition_id_tensor())
        outs = b2j._bass_exec_p.bind(
            *operands,
            out_avals=tuple(out_avals),
            in_names=tuple(all_in),
            out_names=tuple(out_names),
            lowering_input_output_aliases=(),
            sim_require_finite=True,
            sim_require_nnan=True,
            nc=nc,
        )
        return tuple(outs)

    devices = jax.devices()[:n_cores]
    mesh = Mesh(np.asarray(devices), ("core",))
    in_specs = (PartitionSpec("core"),) * (n_params + n_outs)
    out_specs = (PartitionSpec("core"),) * n_outs
    # The kernel writes every element of every output, so the zero
    # "output seed" operands are never read: keep them non-donated and
    # reuse one cached device-resident zeros set for every call.
    sharded = jax.jit(
        shard_map(_body, mesh=mesh, in_specs=in_specs,
                  out_specs=out_specs, check_rep=False),
        keep_unused=True)
    shardings = tuple(NamedSharding(mesh, PartitionSpec("core"))
                      for _ in out_avals)
    zero_factory = jax.jit(
        lambda: tuple(
            jnp.zeros((n_cores * a.shape[0], *a.shape[1:]), a.dtype)
            for a in out_avals),
        out_shardings=shardings)
    info = (in_names, out_names, out_avals, sharded, zero_factory)
    _runner_cache[key] = info
    return info


_concat_cache = {}
_zeros_next = {}


def run_spmd(nc, in_maps, n_cores=NCORES):
    """Run nc on n_cores; returns {out_name: [n_cores, *shape] ndarray}."""
    in_names, out_names, out_avals, sharded, zero_factory = _get_runner(
        nc, n_cores)
    ckey = (id(nc), id(in_maps))
    concat_in = _concat_cache.get(ckey)
    if concat_in is None:
        concat_in = [
            np.concatenate(
                [np.asarray(in_maps[c][n]) for c in range(n_cores)], axis=0)
            for n in in_names
        ]
        _concat_cache[ckey] = concat_in
        if len(_concat_cache) > 4:
            _concat_cache.pop(next(iter(_concat_cache)))
    zeros = _zeros_next.get(id(nc))
    if zeros is None:
        zeros = zero_factory()
        _zeros_next[id(nc)] = zeros
    out_arrs = sharded(*concat_in, *zeros)
    return {
        n: np.asarray(out_arrs[i]).reshape(n_cores, *out_avals[i].shape)
        for i, n in enumerate(out_names)
    }


def kernel(x, edge_index, Ws, att_src, att_dst, biases):
    nc, in_maps = make_program_and_inputs(
        x, edge_index, Ws, att_src, att_dst, biases)
    res = run_spmd(nc, in_maps)
    out = res["out"].reshape(NCORES * NSH, D).astype(np.float32)
    return out[:N_NODES]


# revision 30
# speedup vs baseline: 1.0310x; 1.0310x over previous
"""GAT (8-layer, 8-head) Trainium2 Bass kernel, 8-core SPMD — v2.

Empirical cost model of this (axon-tunneled) environment, measured:
per-call wall = dispatch floor (~0.1s) + bytes-through-tunnel (~125MB/s)
+ ~35us per STATIC instruction (per-call program processing); dynamic
(executed) instructions are nearly free, so hardware For_i loops +
minimal I/O dominate everything a classic roofline would suggest.

v2 therefore:
- rolls the per-layer window loop into a hardware For_i (one ~56-instr
  static body instead of 8x20 unrolled copies),
- unrolls the 8 layers (collective_compute inside For_i hangs at runtime),
- phase A emits node-major [h|s] directly (lhsT = x^T chunk stationary),
  with batched PSUM evacuation,
- minimizes shipped bytes: x and out in bf16; only src indices (int16)
  and window-local dst offsets (int8) are shipped, 16-partition-wide,
  replicated on device; dst gather indices and the one-hot dloc table
  are derived on device (each layer copies the own shard to a fixed
  table slab so dst rows are core-independent); weights/biases sharded
  8-ways and reassembled with an AllGather; iota/identity generated on
  device,
- caches the traced jit executable + concatenated inputs across calls and
  generates donated output zeros on device (run_bass_via_pjrt re-traces
  and ships host zeros every call).
Hard limits found empirically: dma_gather >1024 indices per call crashes
(runtime descriptor ring), dma_scatter_add races on duplicate indices,
matmul stationary (ldweights) cannot take register offsets, compute
engines cannot start at partition offsets not multiple of 32.

Math identical to v1: fused [h|s] = x @ [W | W@A2], AllGather full node
table, per dst-window gather rows by src and dst, e=lrelu(s_src+s_dst),
ex=exp(e) (no max subtraction), one-hot scatter via PE matmul
psum[j,72] += S^T [ex*h | ex], out = psum[:,:64]/(psum[:,64:72]+1e-16)+b.
"""

import numpy as np
import ml_dtypes

N_NODES = 20000
N_EDGES = 640000
L, H, C = 8, 8, 8
D = H * C  # 64
NEG_SLOPE = 0.2

NCORES = 8
WIN = 128                 # dst nodes per window
WPC = 20                  # windows per core
NSH = WIN * WPC           # 2560 nodes per shard
NPAD = NCORES * NSH       # 20480
SENT = NPAD               # sentinel node id (table row)
TROWS = NPAD + 1          # table rows (incl. sentinel)

_cache = {}
USE_LOOP = True     # window loop as HW For_i (static-size win)
FAT_GATHER = False  # >1024-idx dma_gather crashes the device; keep chunked
BATCH_PST = True    # batched PSUM evacuation in phase A


# ----------------------------------------------------------------------------
# Host preprocessing
# ----------------------------------------------------------------------------
def _prep_edges(edge_index):
    src = np.asarray(edge_index[0], dtype=np.int64)
    dst = np.asarray(edge_index[1], dtype=np.int64)
    src = np.concatenate([src, np.arange(N_NODES, dtype=np.int64)])
    dst = np.concatenate([dst, np.arange(N_NODES, dtype=np.int64)])
    order = np.argsort(dst, kind="stable")
    src, dst = src[order], dst[order]

    nwin = NCORES * WPC  # 160
    win_of_edge = dst // WIN
    counts = np.bincount(win_of_edge, minlength=nwin)
    tpw = int(np.ceil(counts.max() / 128))
    nsw = tpw * 128                      # slots per window
    nslot = WPC * nsw                    # slots per core

    src_slot = np.full((NCORES, nslot), SENT, dtype=np.int64)
    dst_slot = np.full((NCORES, nslot), SENT, dtype=np.int64)
    dloc_slot = np.full((NCORES, nslot), -1.0, dtype=np.float32)

    wstart = np.zeros(nwin + 1, dtype=np.int64)
    np.cumsum(counts, out=wstart[1:])
    for w in range(nwin):
        c, wl = divmod(w, WPC)
        e0, e1 = wstart[w], wstart[w + 1]
        s0 = wl * nsw
        n = e1 - e0
        src_slot[c, s0:s0 + n] = src[e0:e1]
        dst_slot[c, s0:s0 + n] = dst[e0:e1]
        dloc_slot[c, s0:s0 + n] = (dst[e0:e1] - w * WIN).astype(np.float32)

    def wrap16(a, dt):
        # index i -> [i%16, i//16]; replicated to 128 partitions on device
        return a.reshape(-1, 16).T.astype(dt).copy()   # [16, nslot/16]

    srcidx = np.stack([wrap16(src_slot[c], np.int16) for c in range(NCORES)])
    dl16 = np.stack([wrap16(dloc_slot[c].astype(np.int64), np.int8)
                     for c in range(NCORES)])
    return tpw, srcidx, dl16


# ----------------------------------------------------------------------------
# Bass program
# ----------------------------------------------------------------------------
def _build(tpw):
    import concourse.bass as bass
    import concourse.tile as tile
    import concourse.mybir as mybir
    from concourse import bacc
    from contextlib import ExitStack

    f32 = mybir.dt.float32
    bf16 = mybir.dt.bfloat16
    i16 = mybir.dt.int16
    Alu = mybir.AluOpType
    Act = mybir.ActivationFunctionType

    nsw = tpw * 128
    nslot = WPC * nsw

    nc = bacc.Bacc("TRN2", target_bir_lowering=False, debug=False,
                   num_devices=NCORES)

    # external I/O
    t_xsh = nc.dram_tensor("xsh", [NSH, D], bf16, kind="ExternalInput")
    t_srci = nc.dram_tensor("srcidx", [16, nslot // 16], i16, kind="ExternalInput")
    t_dl16 = nc.dram_tensor("dl16", [16, nslot // 16], mybir.dt.int8,
                            kind="ExternalInput")
    t_wts = nc.dram_tensor("wts", [8, L, 80], f32, kind="ExternalInput")
    t_brep = nc.dram_tensor("brep", [2, L, 64], f32, kind="ExternalInput")
    t_out = nc.dram_tensor("out", [NSH, D], bf16, kind="ExternalOutput")

    with tile.TileContext(nc) as tc, ExitStack() as ctx:
        cpool = ctx.enter_context(tc.tile_pool(name="const", bufs=1))
        gpool = ctx.enter_context(tc.tile_pool(name="gath", bufs=2))
        epool = ctx.enter_context(tc.tile_pool(name="edge", bufs=2))
        dram = ctx.enter_context(tc.tile_pool(name="dram", bufs=1, space="DRAM"))
        psT = ctx.enter_context(tc.tile_pool(name="psT", bufs=2, space="PSUM"))
        psA = ctx.enter_context(tc.tile_pool(name="psA", bufs=2, space="PSUM"))
        psW = ctx.enter_context(tc.tile_pool(name="psW", bufs=2, space="PSUM"))

        # persistent SBUF
        sb_x = cpool.tile([128, WPC, D], f32)          # node-major shard x
        sb_srci = cpool.tile([128, WPC, nsw // 16], i16)
        sb_dl16 = cpool.tile([128, WPC, nsw // 16], mybir.dt.int8)
        sb_dloc = cpool.tile([128, WPC, tpw], bf16)
        sb_iota = cpool.tile([128, 128], bf16)
        sb_ident = cpool.tile([128, 128], f32)
        sb_wts = cpool.tile([64, L, 80], f32)
        sb_brep = cpool.tile([128, L, 64], f32)
        xT = cpool.tile([64, WPC, 128], f32)           # feature-major shard x
        tabsb = cpool.tile([128, WPC, 128], bf16)

        xin = cpool.tile([128, WPC, D], bf16)
        nc.sync.dma_start(xin[:], t_xsh.ap().rearrange("(t p) c -> p t c", p=128))
        nc.vector.tensor_copy(sb_x[:], xin[:])
        for r in range(8):
            nc.sync.dma_start(sb_srci[16 * r:16 * (r + 1), :, :],
                              t_srci.ap().rearrange("p (w s) -> p w s", w=WPC))
            nc.sync.dma_start(sb_dl16[16 * r:16 * (r + 1), :, :],
                              t_dl16.ap().rearrange("p (w s) -> p w s", w=WPC))
        # wrap128 dloc from the wrap16 table:
        # wrap128[16*(q%8)+r, j] = wrap16[r, 8j+(q%8)]  (replicated blocks)
        dl8 = cpool.tile([128, WPC, tpw], mybir.dt.int8)
        dl16v = sb_dl16[:].rearrange("p w (t e) -> p w t e", e=8)
        for pb in range(8):
            nc.sync.dma_start(dl8[16 * pb:16 * (pb + 1), :, :],
                              dl16v[16 * pb:16 * (pb + 1), :, :, pb])
        nc.vector.tensor_copy(sb_dloc[:], dl8[:])
        # iota row (0..127 along free) and identity, generated on device
        irow = cpool.tile([128, 128], i16)
        icol = cpool.tile([128, 128], i16)
        nc.gpsimd.iota(irow[:], pattern=[[1, 128]], base=0, channel_multiplier=0)
        nc.gpsimd.iota(icol[:], pattern=[[0, 128]], base=0, channel_multiplier=1)
        nc.vector.tensor_copy(sb_iota[:], irow[:])
        nc.vector.tensor_tensor(sb_ident[:], irow[:], icol[:], Alu.is_equal)
        # commons are sharded across cores; AllGather reassembles them
        WFULL = dram.tile([64, L, 80], f32)
        BFULL = dram.tile([16, L, 64], f32)
        WSTG = dram.tile([8, L, 80], f32)
        BSTG = dram.tile([2, L, 64], f32)
        nc.sync.dma_start(WSTG[:], t_wts.ap())
        nc.sync.dma_start(BSTG[:], t_brep.ap())
        nc.gpsimd.collective_compute(
            "AllGather", Alu.bypass, replica_groups=[list(range(NCORES))],
            ins=[WSTG[:].opt()], outs=[WFULL[:].opt()])
        nc.gpsimd.collective_compute(
            "AllGather", Alu.bypass, replica_groups=[list(range(NCORES))],
            ins=[BSTG[:].opt()], outs=[BFULL[:].opt()])
        nc.sync.dma_start(sb_wts[:], WFULL[:])
        for r in range(8):
            nc.sync.dma_start(sb_brep[16 * r:16 * (r + 1), :, :], BFULL[:])

        # DRAM: gather table + staging shard. bf16 rows (256B):
        # [0:64] h bf16; f32 view: [32:40] s_src, [40:48] s_dst, [48:64] pad
        TAB = dram.tile([TROWS + NSH, 128], bf16)
        STAGE = dram.tile([NSH, 128], bf16)

        # STAGE bf16 cols 96:128 (f32 48:64) are never produced; zero once
        zjunk = cpool.tile([128, WPC, 32], bf16)
        nc.vector.memset(zjunk[:], 0.0)
        nc.sync.dma_start(
            STAGE[:, 96:128].rearrange("(t p) c -> p t c", p=128), zjunk[:])

        # sentinel row: h=0, s_src=-1e30 (=> ex = 0 for padding), s_dst=0
        sent = cpool.tile([1, 128], bf16)
        nc.vector.memset(sent[:], 0.0)
        nc.vector.memset(sent[:].bitcast(f32)[:, 32:40], -1e30)
        nc.sync.dma_start(TAB[SENT:SENT + 1, :], sent[:])

        # dst rows live in the fixed own-slab at TAB[TROWS + 128*w + dloc]
        iw = cpool.tile([128, WPC], i16)
        nc.gpsimd.iota(iw[:], pattern=[[128, WPC]], base=TROWS,
                       channel_multiplier=0)
        sb_ofsf = cpool.tile([128, WPC, 1], f32)
        nc.vector.tensor_copy(sb_ofsf[:].rearrange("p w e -> p (w e)"), iw[:])

        for l in range(L):
            # ---------------- phase A: per-node prep (own shard) ----------
            # x^T chunks via PE transpose (batched psum evac, 4 per bank)
            if BATCH_PST:
                for t0 in range(0, WPC, 4):
                    pt = psT.tile([64, 4, 128], f32, tag="psT")
                    for j in range(4):
                        nc.tensor.transpose(pt[:, j, :], sb_x[:, t0 + j, :],
                                            sb_ident[:])
                    nc.scalar.copy(xT[:, t0:t0 + 4, :], pt[:])
            else:
                for t in range(WPC):
                    pt = psT.tile([64, 128], f32, tag="psT")
                    nc.tensor.transpose(pt[:], sb_x[:, t, :], sb_ident[:])
                    nc.scalar.copy(xT[:, t, :], pt[:])

            # node-major [h|s] directly: out[128n, 80] = xT_chunk^T @ wts_l
            if BATCH_PST:
                for t0 in range(0, WPC, 5):
                    ph = psA.tile([128, 5, 80], f32, tag="psA")
                    for j in range(5):
                        nc.tensor.matmul(ph[:, j, :], lhsT=xT[:, t0 + j, :],
                                         rhs=sb_wts[:, l, :],
                                         start=True, stop=True)
                    nc.scalar.copy(tabsb[:, t0:t0 + 5, 0:64], ph[:, :, 0:64])
                    nc.vector.tensor_copy(
                        tabsb[:, t0:t0 + 5, :].bitcast(f32)[:, :, 32:48],
                        ph[:, :, 64:80])
            else:
                for t in range(WPC):
                    ph = psA.tile([128, 80], f32, tag="psA")
                    nc.tensor.matmul(ph[:], lhsT=xT[:, t, :],
                                     rhs=sb_wts[:, l, :],
                                     start=True, stop=True)
                    nc.scalar.copy(tabsb[:, t, 0:64], ph[:, 0:64])
                    nc.vector.tensor_copy(
                        tabsb[:, t, :].bitcast(f32)[:, 32:48], ph[:, 64:80])

            nc.sync.dma_start(
                STAGE[:, 0:96].rearrange("(t p) c -> p t c", p=128),
                tabsb[:, :, 0:96])
            nc.sync.dma_start(
                TAB[TROWS:TROWS + NSH, :].rearrange("(t p) c -> p t c", p=128),
                tabsb[:])
            nc.gpsimd.collective_compute(
                "AllGather", Alu.bypass,
                replica_groups=[list(range(NCORES))],
                ins=[STAGE[:].opt()],
                outs=[TAB[0:NPAD, :].opt()],
            )

            # ---------------- phase B: edges, per window (HW loop) --------
            from contextlib import nullcontext

            def _window_body(w):
                vs = gpool.tile([128, tpw, 128], bf16, tag="vsrc")
                vd = gpool.tile([128, tpw, 128], bf16, tag="vdst")
                vdix = epool.tile([128, nsw // 16], i16, tag="vdix")
                nc.vector.tensor_scalar_add(vdix[:], sb_dl16[:, w, :],
                                            sb_ofsf[:, w, :])

                if FAT_GATHER:
                    nc.gpsimd.dma_gather(
                        out_ap=vs[:], in_ap=TAB[:], idxs_ap=sb_srci[:, w, :],
                        num_idxs=nsw, num_idxs_reg=nsw, elem_size=128)
                    nc.gpsimd.dma_gather(
                        out_ap=vd[:], in_ap=TAB[:], idxs_ap=vdix[:],
                        num_idxs=nsw, num_idxs_reg=nsw, elem_size=128)
                else:
                    GCH = 8
                    for u0 in range(0, tpw, GCH):
                        u1 = min(u0 + GCH, tpw)
                        n = (u1 - u0) * 128
                        nc.gpsimd.dma_gather(
                            out_ap=vs[:, u0:u1, :], in_ap=TAB[:],
                            idxs_ap=sb_srci[:, w, u0 * 8:u1 * 8],
                            num_idxs=n, num_idxs_reg=n, elem_size=128)
                        nc.gpsimd.dma_gather(
                            out_ap=vd[:, u0:u1, :], in_ap=TAB[:],
                            idxs_ap=vdix[:, u0 * 8:u1 * 8],
                            num_idxs=n, num_idxs_reg=n, elem_size=128)

                # one-hot S: [128, tpw, 128] bf16
                S = epool.tile([128, tpw, 128], bf16, tag="S")
                nc.vector.tensor_tensor(
                    S[:],
                    sb_dloc[:, w, :].unsqueeze(2).broadcast_to([128, tpw, 128]),
                    sb_iota[:].unsqueeze(1).broadcast_to([128, tpw, 128]),
                    Alu.is_equal)

                # e = lrelu(s_src + s_dst); ex = exp(e)
                e = epool.tile([128, tpw, 8], f32, tag="e")
                nc.vector.tensor_tensor(
                    e[:], vs[:].bitcast(f32)[:, :, 32:40],
                    vd[:].bitcast(f32)[:, :, 40:48], Alu.add)
                nc.vector.scalar_tensor_tensor(e[:], e[:], NEG_SLOPE, e[:],
                                               op0=Alu.mult, op1=Alu.max)
                ex = epool.tile([128, tpw, 8], f32, tag="ex")
                nc.scalar.activation(ex[:], e[:], Act.Exp)
                # R = [V*ex | ex] in bf16
                R = epool.tile([128, tpw, 72], bf16, tag="R")
                nc.vector.tensor_copy(R[:, :, 64:72], ex[:])
                nc.vector.tensor_tensor(
                    R[:, :, 0:64].rearrange("p t (h c) -> p t h c", h=8),
                    vs[:, :, 0:64].rearrange("p t (h c) -> p t h c", h=8),
                    R[:, :, 64:72].unsqueeze(3).broadcast_to(
                        [128, tpw, 8, 8]),
                    Alu.mult)

                pw = psW.tile([128, 72], f32, tag="psW")
                for t in range(tpw):
                    nc.tensor.matmul(pw[:], lhsT=S[:, t, :], rhs=R[:, t, :],
                                     start=(t == 0), stop=(t == tpw - 1))

                # out = psum[:, :64] / (z + 1e-16) + bias
                zi = epool.tile([128, 8], f32, tag="zi")
                nc.vector.tensor_scalar_add(zi[:], pw[:, 64:72], 1e-16)
                rz = epool.tile([128, 8], f32, tag="rz")
                nc.vector.reciprocal(rz[:], zi[:])
                xm = epool.tile([128, 64], f32, tag="xm")
                nc.vector.tensor_tensor(
                    xm[:].rearrange("p (h c) -> p h c", h=8),
                    pw[:, 0:64].rearrange("p (h c) -> p h c", h=8),
                    rz[:].unsqueeze(2).broadcast_to([128, 8, 8]),
                    Alu.mult)
                nc.vector.tensor_tensor(sb_x[:, w, :], xm[:],
                                        sb_brep[:, l, :], Alu.add)

            if USE_LOOP:
                with tc.For_i(0, WPC) as w:
                    _window_body(w)
            else:
                for w in range(WPC):
                    _window_body(w)

        outbf = cpool.tile([128, WPC, D], bf16)
        nc.vector.tensor_copy(outbf[:], sb_x[:])
        nc.sync.dma_start(t_out.ap().rearrange("(t p) c -> p t c", p=128),
                          outbf[:])

    nc.finalize()
    return nc


def _get_program(tpw):
    key = (tpw, USE_LOOP, FAT_GATHER, BATCH_PST)
    if key not in _cache:
        _cache[key] = _build(tpw)
    return _cache[key]


# ----------------------------------------------------------------------------
# Entry point
# ----------------------------------------------------------------------------
def make_program_and_inputs(x, edge_index, Ws, att_src, att_dst, biases):
    x = np.asarray(x, dtype=np.float32)
    Ws = np.asarray(Ws, dtype=np.float32)
    att_src = np.asarray(att_src, dtype=np.float32)
    att_dst = np.asarray(att_dst, dtype=np.float32)
    biases = np.asarray(biases, dtype=np.float32)

    tpw, srcidx, dl16 = _prep_edges(edge_index)
    nc = _get_program(tpw)

    xpad = np.zeros((NPAD, D), np.float32)
    xpad[:N_NODES] = x

    # A2[cout, l, 0:8] = att_src heads, [.., 8:16] = att_dst heads
    a2 = np.zeros((64, L, 16), np.float32)
    for l in range(L):
        for h in range(H):
            a2[h * C:(h + 1) * C, l, h] = att_src[l, h]
            a2[h * C:(h + 1) * C, l, 8 + h] = att_dst[l, h]
    # wts[cin, l, 0:64] = W; [cin, l, 64:80] = W @ A2  (s = x @ (W A2))
    wts = np.zeros((64, L, 80), np.float32)
    for l in range(L):
        wts[:, l, 0:64] = Ws[l]
        wts[:, l, 64:80] = Ws[l] @ a2[:, l, :]
    brep16 = np.broadcast_to(biases[None, :, :], (16, L, 64))
    in_maps = []
    for c in range(NCORES):
        in_maps.append(dict(
            wts=np.ascontiguousarray(wts[8 * c:8 * (c + 1)]),
            brep=np.ascontiguousarray(brep16[2 * c:2 * (c + 1)]),
            xsh=np.ascontiguousarray(
                xpad[c * NSH:(c + 1) * NSH]).astype(ml_dtypes.bfloat16),
            srcidx=srcidx[c], dl16=dl16[c]))
    return nc, in_maps


# ----------------------------------------------------------------------------
# Cached PJRT runner. Mirrors concourse.bass2jax.run_bass_via_pjrt's
# multi-core branch, but (a) caches the traced/jitted executable across
# calls (run_bass_via_pjrt rebuilds closures and re-traces every call) and
# (b) generates the donated zero output buffers ON DEVICE instead of
# shipping host zeros through the axon tunnel each call.
# ----------------------------------------------------------------------------
_runner_cache = {}


def _get_runner(nc, n_cores):
    key = id(nc)
    if key in _runner_cache:
        return _runner_cache[key]
    import jax
    import jax.numpy as jnp
    from jax.experimental.shard_map import shard_map
    from jax.sharding import Mesh, NamedSharding, PartitionSpec
    from concourse import bass2jax as b2j
    import concourse.mybir as mybir

    b2j.install_neuronx_cc_hook()
    partition_name = (nc.partition_id_tensor.name
                      if nc.partition_id_tensor else None)
    in_names, out_names, out_avals = [], [], []
    for alloc in nc.m.functions[0].allocations:
        if not isinstance(alloc, mybir.MemoryLocationSet):
            continue
        name = alloc.memorylocations[0].name
        if alloc.kind == "ExternalInput":
            if name != partition_name:
                in_names.append(name)
        elif alloc.kind == "ExternalOutput":
            shape = tuple(alloc.tensor_shape)
            dtype = mybir.dt.np(alloc.dtype)
            out_names.append(name)
            out_avals.append(jax.core.ShapedArray(shape, dtype))
    n_params = len(in_names)
    n_outs = len(out_names)
    all_in = in_names + out_names + ([partition_name] if partition_name else [])

    def _body(*args):
        operands = list(args)
        if partition_name is not None:
            operands.append(b2j.partition_id_tensor())
        outs = b2j._bass_exec_p.bind(
            *operands,
            out_avals=tuple(out_avals),
            in_names=tuple(all_in),
            out_names=tuple(out_names),
            lowering_input_output_aliases=(),
            sim_require_finite=True,
            sim_require_nnan=True,
            nc=nc,
        )
        return tuple(outs)

    devices = jax.devices()[:n_cores]
    mesh = Mesh(np.asarray(devices), ("core",))
    in_specs = (PartitionSpec("core"),) * (n_params + n_outs)
    out_specs = (PartitionSpec("core"),) * n_outs
    # The kernel writes every element of every output, so the zero
    # "output seed" operands are never read: keep them non-donated and
    # reuse one cached device-resident zeros set for every call.
    sharded = jax.jit(
        shard_map(_body, mesh=mesh, in_specs=in_specs,
                  out_specs=out_specs, check_rep=False),
        keep_unused=True)
    sharding = NamedSharding(mesh, PartitionSpec("core"))
    shardings = tuple(sharding for _ in out_avals)
    zero_factory = jax.jit(
        lambda: tuple(
            jnp.zeros((n_cores * a.shape[0], *a.shape[1:]), a.dtype)
            for a in out_avals),
        out_shardings=shardings)
    info = (in_names, out_names, out_avals, sharded, zero_factory, sharding)
    _runner_cache[key] = info
    return info


_concat_cache = {}
_zeros_next = {}


def run_spmd(nc, in_maps, n_cores=NCORES):
    """Run nc on n_cores; returns {out_name: [n_cores, *shape] ndarray}."""
    import jax
    in_names, out_names, out_avals, sharded, zero_factory, sharding = \
        _get_runner(nc, n_cores)
    ckey = (id(nc), id(in_maps))
    concat_in = _concat_cache.get(ckey)
    if concat_in is None:
        concat_in = [
            np.concatenate(
                [np.asarray(in_maps[c][n]) for c in range(n_cores)], axis=0)
            for n in in_names
        ]
        _concat_cache[ckey] = concat_in
        if len(_concat_cache) > 4:
            _concat_cache.pop(next(iter(_concat_cache)))
    zeros = _zeros_next.get(id(nc))
    if zeros is None:
        zeros = zero_factory()
        _zeros_next[id(nc)] = zeros
    # launch all host->device transfers asynchronously up front
    dev_in = [jax.device_put(a, sharding) for a in concat_in]
    out_arrs = sharded(*dev_in, *zeros)
    return {
        n: np.asarray(out_arrs[i]).reshape(n_cores, *out_avals[i].shape)
        for i, n in enumerate(out_names)
    }


def kernel(x, edge_index, Ws, att_src, att_dst, biases):
    nc, in_maps = make_program_and_inputs(
        x, edge_index, Ws, att_src, att_dst, biases)
    res = run_spmd(nc, in_maps)
    out = res["out"].reshape(NCORES * NSH, D).astype(np.float32)
    return out[:N_NODES]
